# revision 12
# baseline (speedup 1.0000x reference)
"""Multi-head causal attention with RoPE for TRN2, sharded over 8 NeuronCores.

Sharding: core = b*4 + g  (b in {0,1} batches, g in {0..3} head groups of 4
heads).  Each core computes qkv projection for its batch restricted to its
heads, RoPE, attention, and a partial output projection (row-slice of w_out).
Host sums the 4 partial outputs per batch.

Device layout notes:
  - x is passed TRANSPOSED (xT [E, S]) so q^T/k^T (head-dim on partitions)
    and V (seq on partitions) all come straight out of matmuls.
  - q^T/k^T are stored as two 128-row tiles: A = first rotary halves of the
    4 heads (4 x 32 rows), B = second halves.  RoPE is then 6 full-width
    elementwise ops per 512-seq chunk.
  - Scores are computed transposed (S^T[sk, sq] = k^T.T @ q^T) so that after
    exp, P^T is exactly the moving operand for the A·V matmul (no on-chip
    transposes anywhere).
  - Softmax has no max-subtraction: scaled scores are ~N(0,1) (bounded ~±8)
    so exp cannot overflow fp32.  Row sums l come for free from a ones
    column appended to V (M=65 stationary).  The 1/l normalisation is
    applied to y^T via a partition-broadcast DMA of 1/l.
  - Causality: matmul N-ranges are trimmed to the valid suffix per 128-key
    block; the single partial (diagonal) 128x128 block is masked by a
    0/1 lower-triangle multiply on the gpsimd engine after exp.
"""

import os

import numpy as np

import concourse.bass as bass
from concourse import bacc
import concourse.mybir as mybir
import concourse.tile as tile
from concourse.tile_rust import add_dep_helper
from concourse.bass_utils import run_bass_kernel_spmd

F32 = mybir.dt.float32
F32R = mybir.dt.float32r
AF = mybir.ActivationFunctionType

B, S, E = 2, 2048, 1024
H, D = 16, 64
NCORE = 8
G = 4  # head groups (cores per batch)
HPG = 4  # heads per group
SC = 512  # seq chunk
NSC = S // SC  # 4
NST = S // 128  # 16 key blocks
KT = E // 128  # contraction tiles for projections
SCALE = 1.0 / np.sqrt(D)

LAST_RESULT = None  # BassKernelResults of the most recent run (for test.py)


def _r(ap):
    return ap.bitcast(F32R)


def build_nc(is_causal: bool):
    nc = bacc.Bacc("TRN2", debug=False)
    xT = nc.dram_tensor("xT", [E, S], F32, kind="ExternalInput").ap()
    wqA = nc.dram_tensor("wqA", [E, 128], F32, kind="ExternalInput").ap()
    wqB = nc.dram_tensor("wqB", [E, 128], F32, kind="ExternalInput").ap()
    wkA = nc.dram_tensor("wkA", [E, 128], F32, kind="ExternalInput").ap()
    wkB = nc.dram_tensor("wkB", [E, 128], F32, kind="ExternalInput").ap()
    wv = nc.dram_tensor("wv", [E, HPG * D], F32, kind="ExternalInput").ap()
    wout = nc.dram_tensor("wout", [HPG * D, E], F32, kind="ExternalInput").ap()
    cosb = nc.dram_tensor("cosb", [128, S], F32, kind="ExternalInput").ap()
    sinb = nc.dram_tensor("sinb", [128, S], F32, kind="ExternalInput").ap()
    tri = kbias = None
    if is_causal:
        tri = nc.dram_tensor("tri", [128, 128], F32, kind="ExternalInput").ap()
    else:
        kbias = nc.dram_tensor("kbias", [S], F32, kind="ExternalInput").ap()
    outp = nc.dram_tensor("outp", [S, E], F32, kind="ExternalOutput").ap()
    rl_dram = nc.dram_tensor("rl_scratch", [NSC, HPG, SC], F32).ap()

    from contextlib import ExitStack

    with tile.TileContext(nc) as tc, ExitStack() as ctx:
        _trace(
            ctx, tc, xT, wqA, wqB, wkA, wkB, wv, wout, cosb, sinb, tri, kbias, outp,
            rl_dram, is_causal,
        )
    nc.compile()
    return nc


def _trace(ctx, tc, xT, wqA, wqB, wkA, wkB, wv, wout, cosb, sinb, tri, kbias, outp,
           rl_dram, is_causal):
    nc = tc.nc
    persist = ctx.enter_context(tc.tile_pool(name="persist", bufs=1))

    cos_sb = persist.tile([128, S], F32)
    nc.sync.dma_start(cos_sb, cosb)
    sin_sb = persist.tile([128, S], F32)
    nc.sync.dma_start(sin_sb, sinb)
    if is_causal:
        tri_sb = persist.tile([128, 128], F32)
        nc.sync.dma_start(tri_sb, tri)
    else:
        kb_sb = persist.tile([128, NST], F32)
        nc.sync.dma_start(kb_sb, kbias.rearrange("(t p) -> p t", p=128))

    wqA_sb = persist.tile([128, KT, 128], F32R)
    wqB_sb = persist.tile([128, KT, 128], F32R)
    wkA_sb = persist.tile([128, KT, 128], F32R)
    wkB_sb = persist.tile([128, KT, 128], F32R)
    wv_sb = persist.tile([128, KT, HPG * D], F32R)
    wout_sb = persist.tile([128, 2, E], F32R)

    qTA_sb = persist.tile([128, S], F32R)
    qTB_sb = persist.tile([128, S], F32R)
    kTA_sb = persist.tile([128, S], F32R)
    kTB_sb = persist.tile([128, S], F32R)
    # V with a ones column appended per head: [s-block, head, 65]
    v_sb = persist.tile([128, NST, HPG, D + 1], F32R)
    ones_c = persist.tile([128, 1], F32)
    nc.vector.memset(ones_c, 1.0)
    nc.vector.tensor_copy(
        v_sb[:, :, :, D : D + 1], ones_c.to_broadcast((128, NST, HPG, 1))
    )

    # ---------------- phase 1: qkv projection + rope ----------------
    from contextlib import ExitStack

    p1ctx = ctx.enter_context(ExitStack())
    xpool = p1ctx.enter_context(tc.tile_pool(name="xt", bufs=1))
    xrpool = p1ctx.enter_context(tc.tile_pool(name="xtr", bufs=2))
    p1ps = p1ctx.enter_context(tc.tile_pool(name="p1ps", bufs=1, space="PSUM"))
    rtmp = p1ctx.enter_context(tc.tile_pool(name="rtmp", bufs=2))

    wstage = p1ctx.enter_context(tc.tile_pool(name="wstage", bufs=1))
    for dram, dst, shp in (
        (wqA, wqA_sb, [128, KT, 128]),
        (wqB, wqB_sb, [128, KT, 128]),
        (wkA, wkA_sb, [128, KT, 128]),
        (wkB, wkB_sb, [128, KT, 128]),
        (wv, wv_sb, [128, KT, HPG * D]),
        (wout, wout_sb, [128, 2, E]),
    ):
        wstg = wstage.tile(shp, F32, name="wstg", tag=dram.name)
        nc.sync.dma_start(wstg, dram.rearrange("(t p) m -> p t m", p=128))
        nc.vector.tensor_copy(dst, wstg)

    xT_t = xT.rearrange("(t p) s -> p t s", p=128)
    for c in range(NSC):
        sl = slice(c * SC, (c + 1) * SC)
        xt = xpool.tile([128, KT, SC], F32, name="xt", tag="xt")
        nc.sync.dma_start(xt, xT_t[:, :, sl])
        xtr = xrpool.tile([128, KT, SC], F32R, name="xtr", tag="xtr")
        nc.gpsimd.tensor_copy(xtr, xt)

        qa_ps = p1ps.tile([128, SC], F32, name="qa_ps", tag="qa")
        qb_ps = p1ps.tile([128, SC], F32, name="qb_ps", tag="qb")
        ka_ps = p1ps.tile([128, SC], F32, name="ka_ps", tag="ka")
        kb_ps = p1ps.tile([128, SC], F32, name="kb_ps", tag="kb")
        for t in range(KT):
            st, sp = (t == 0), (t == KT - 1)
            nc.tensor.matmul(qa_ps, wqA_sb[:, t, :], xtr[:, t, :], start=st, stop=sp)
            nc.tensor.matmul(qb_ps, wqB_sb[:, t, :], xtr[:, t, :], start=st, stop=sp)
            nc.tensor.matmul(ka_ps, wkA_sb[:, t, :], xtr[:, t, :], start=st, stop=sp)
            nc.tensor.matmul(kb_ps, wkB_sb[:, t, :], xtr[:, t, :], start=st, stop=sp)
        for u in range(SC // 128):
            vps = p1ps.tile([128, HPG * D], F32, name="vps", tag="vps", bufs=2)
            for t in range(KT):
                nc.tensor.matmul(
                    vps, xtr[:, t, u * 128 : (u + 1) * 128], wv_sb[:, t, :],
                    start=(t == 0), stop=(t == KT - 1),
                )
            nc.vector.tensor_copy(
                v_sb[:, c * (SC // 128) + u, :, 0:D],
                vps.rearrange("p (h d) -> p h d", h=HPG),
            )

        cs, sn = cos_sb[:, sl], sin_sb[:, sl]
        for aps, bps, asb, bsb in (
            (qa_ps, qb_ps, qTA_sb, qTB_sb),
            (ka_ps, kb_ps, kTA_sb, kTB_sb),
        ):
            t1 = rtmp.tile([128, SC], F32, name="t1", tag="t1")
            t2 = rtmp.tile([128, SC], F32, name="t2", tag="t2")
            nc.vector.tensor_mul(t1, aps, cs)
            nc.vector.tensor_mul(t2, bps, sn)
            nc.vector.tensor_sub(asb[:, sl], t1, t2)
            t3 = rtmp.tile([128, SC], F32, name="t3", tag="t1")
            t4 = rtmp.tile([128, SC], F32, name="t4", tag="t2")
            nc.vector.tensor_mul(t3, bps, cs)
            nc.vector.tensor_mul(t4, aps, sn)
            nc.vector.tensor_add(bsb[:, sl], t3, t4)

    # ---------------- phase 2: attention, phase 3: out projection ----------------
    p1ctx.close()
    p2ps = ctx.enter_context(tc.tile_pool(name="p2ps", bufs=1, space="PSUM"))
    ptpool = ctx.enter_context(tc.tile_pool(name="pt", bufs=2))
    ytsb_pool = ctx.enter_context(tc.tile_pool(name="ytsb", bufs=2))
    opool = ctx.enter_context(tc.tile_pool(name="osb", bufs=2))
    rl_pool = ctx.enter_context(tc.tile_pool(name="rl", bufs=2))

    for c in range(NSC):
        jmax = 4 * c + 4 if is_causal else NST
        yt_ps = [
            p2ps.tile([D + 1, SC], F32, name=f"yt{h}", tag=f"yt{h}")
            for h in range(HPG)
        ]
        for j in range(jmax):
            u = j - 4 * c
            lo = 128 * u if (is_causal and u >= 0) else 0
            st_ps = p2ps.tile([128, HPG, SC], F32, name="st_ps", tag="bigps")
            for h in range(HPG):
                nc.tensor.matmul(
                    st_ps[:, h, lo:],
                    kTA_sb[32 * h : 32 * h + 32, j * 128 : (j + 1) * 128],
                    qTA_sb[32 * h : 32 * h + 32, c * SC + lo : (c + 1) * SC],
                    start=True, stop=False, tile_position=(32 * h, 0),
                )
            for h in range(HPG):
                nc.tensor.matmul(
                    st_ps[:, h, lo:],
                    kTB_sb[32 * h : 32 * h + 32, j * 128 : (j + 1) * 128],
                    qTB_sb[32 * h : 32 * h + 32, c * SC + lo : (c + 1) * SC],
                    start=False, stop=True, tile_position=(32 * h, 0),
                )
            pt = ptpool.tile([128, HPG, SC], F32R, name="pt", tag="pt")
            if is_causal:
                nc.scalar.activation(pt[:, :, lo:], st_ps[:, :, lo:], AF.Exp, scale=SCALE)
            else:
                nc.scalar.activation(
                    pt[:, :, lo:], st_ps[:, :, lo:], AF.Exp,
                    bias=kb_sb[:, j : j + 1], scale=SCALE,
                )
            if is_causal and u >= 0:
                for h in range(HPG):
                    nc.gpsimd.tensor_mul(
                        pt[:, h, lo : lo + 128], pt[:, h, lo : lo + 128], tri_sb
                    )
            for h in range(HPG):
                nc.tensor.matmul(
                    yt_ps[h][:, lo:],
                    v_sb[:, j, h, :],
                    pt[:, h, lo:],
                    start=(j == 0), stop=(j == jmax - 1),
                    skip_group_check=True,
                )

        # normalise y^T by 1/l and stack heads into out-proj lhsT layout
        yt_sb = [
            ytsb_pool.tile([128, SC], F32R, name=f"ytsb{t}", tag=f"ytsb{t}")
            for t in range(2)
        ]
        for h in range(HPG):
            rl = rl_pool.tile([1, SC], F32, name="rl", tag="rl")
            nc.vector.reciprocal(rl, yt_ps[h][D : D + 1, :])
            w_dma = nc.sync.dma_start(rl_dram[c, h].unsqueeze(0), rl)
            rlb = rl_pool.tile([D, SC], F32, name="rlb", tag="rlb")
            r_dma = nc.sync.dma_start(
                rlb, rl_dram[c, h].unsqueeze(0).to_broadcast((D, SC))
            )
            add_dep_helper(r_dma.ins, w_dma.ins, reason="rl dram bounce ordering")
            nc.vector.tensor_mul(
                yt_sb[h // 2][D * (h % 2) : D * (h % 2) + D, :],
                yt_ps[h][0:D, :],
                rlb,
            )
        for u in range(SC // 128):
            out_ps = p2ps.tile([128, E], F32, name="out_ps", tag="bigps")
            for n_ in range(E // 512):
                for t in range(2):
                    nc.tensor.matmul(
                        out_ps[:, n_ * 512 : (n_ + 1) * 512],
                        yt_sb[t][:, u * 128 : (u + 1) * 128],
                        wout_sb[:, t, n_ * 512 : (n_ + 1) * 512],
                        start=(t == 0), stop=(t == 1),
                    )
            o_sb = opool.tile([128, E], F32, name="o_sb", tag="osb")
            nc.vector.tensor_copy(o_sb, out_ps)
            nc.sync.dma_start(outp[c * SC + u * 128 : c * SC + (u + 1) * 128, :], o_sb)


def _rope_tables():
    """cos/sin tables computed with jax on CPU, bit-matching the reference."""
    import jax

    with jax.default_device(jax.devices("cpu")[0]):
        import jax.numpy as jnp

        half = D // 2
        inv_freq = 1.0 / (10000.0 ** (jnp.arange(0, half, dtype=jnp.float32) / half))
        freqs = jnp.arange(S, dtype=jnp.float32)[:, None] * inv_freq[None, :]
        cos = np.asarray(jnp.cos(freqs)).astype(np.float32)  # [S, 32]
        sin = np.asarray(jnp.sin(freqs)).astype(np.float32)
    # broadcast tables for the A/B row layout: row 32*h + d  ->  table col d
    cosb = np.ascontiguousarray(np.tile(cos.T, (HPG, 1)))  # [128, S]
    sinb = np.ascontiguousarray(np.tile(sin.T, (HPG, 1)))
    return cosb, sinb


def kernel(x, attn_mask, w_qkv, w_out, is_causal):
    global LAST_RESULT
    causal = bool(np.asarray(is_causal).item())
    x = np.asarray(x, dtype=np.float32)
    w_qkv = np.asarray(w_qkv, dtype=np.float32)
    w_out = np.asarray(w_out, dtype=np.float32)
    attn_mask = np.asarray(attn_mask).astype(bool)

    nc = build_nc(causal)
    cosb, sinb = _rope_tables()
    tri = np.tril(np.ones((128, 128), dtype=np.float32)).T  # valid: sk <= sq
    in_maps = []
    for core in range(NCORE):
        b, g = divmod(core, G)
        heads = range(HPG * g, HPG * (g + 1))
        m = {
            "xT": np.ascontiguousarray(x[b].T),
            "wqA": np.ascontiguousarray(
                np.concatenate([w_qkv[:, 64 * h : 64 * h + 32] for h in heads], axis=1)
            ),
            "wqB": np.ascontiguousarray(
                np.concatenate([w_qkv[:, 64 * h + 32 : 64 * h + 64] for h in heads], axis=1)
            ),
            "wkA": np.ascontiguousarray(
                np.concatenate(
                    [w_qkv[:, E + 64 * h : E + 64 * h + 32] for h in heads], axis=1
                )
            ),
            "wkB": np.ascontiguousarray(
                np.concatenate(
                    [w_qkv[:, E + 64 * h + 32 : E + 64 * h + 64] for h in heads], axis=1
                )
            ),
            "wv": np.ascontiguousarray(
                np.concatenate(
                    [w_qkv[:, 2 * E + 64 * h : 2 * E + 64 * h + 64] for h in heads],
                    axis=1,
                )
            ),
            "wout": np.ascontiguousarray(
                np.concatenate([w_out[64 * h : 64 * h + 64, :] for h in heads], axis=0)
            ),
            "cosb": cosb,
            "sinb": sinb,
        }
        if causal:
            m["tri"] = tri
        else:
            m["kbias"] = np.where(attn_mask[b], 0.0, -1e30).astype(np.float32)
        in_maps.append(m)

    trace = bool(os.environ.get("ATTN_TRACE"))
    LAST_RESULT = run_bass_kernel_spmd(
        nc, in_maps, list(range(NCORE)), trace=trace
    )
    out = np.zeros((B, S, E), dtype=np.float32)
    for core in range(NCORE):
        b = core // G
        out[b] += LAST_RESULT.results[core]["outp"]
    return out
